# revision 1
# baseline (speedup 1.0000x reference)
"""Trainium2 Bass kernel for nn_CCELoss (calibration-histogram loss).

Sharding: data-parallel over image rows, 8 NeuronCores, 128 rows each.

Per-core layout: logits as [114 = 6 pixel-groups x 19 classes, F=45056]
(group g covers core-flat pixels [g*F, (g+1)*F); tail of group 5 is padding
with logit 0 -> p = 1/19 -> bin 0, corrected on host).

Per 4096-pixel tile:
  ACT  e = exp(l)
  PE   Z[g,n] = sum_c e[(g,c),n]          (block-diag ones matmul, fp32)
  DMA  reshape Z [6,2048] -> [96,128] pixel-major
  ACT  m = ln(Z)                          (Exp/Ln share one ACT table set)
  DMA  reshape back -> [6,2048]
  PE   mb = broadcast m to [114, .]       (block-diag ones matmul)
  DVE  d = l - mb   (in-place over l)
  ACT  p = exp(d)
  folds with fused accumulate, split across DVE and ACT:
    counts N_i = sum [p > i/10]           DVE tensor_scalar(is_gt)
    conf   R_i = sum relu(p - i/10)       DVE max/sub or ACT Relu(bias=-t)
True-class side channel (for the accuracy histogram): host passes the
gathered true-class logit l* in the same pixel-major [96, .] layout;
d* = l* - m, p* = exp(d*) is bit-identical to p at the true class and is
returned to the host, which bins it against target (tiny).
Host: decode folds -> conf/count hists, bin p* -> acc hist, loss formula.
"""

import numpy as np

import bass_rust
import concourse.bass as bass
from concourse import bacc
import concourse.mybir as mybir
import concourse.tile as tile
from concourse.vector_clock import ScopedClock
from concourse.bass_utils import run_bass_kernel_spmd

F32 = mybir.dt.float32
AF = mybir.ActivationFunctionType
ALU = mybir.AluOpType

# ---------------- problem geometry (hardcoded) ----------------
C = 19
NB = 10
H, W = 1024, 2048
NCORES = 8
ROWS = H // NCORES          # 128
NPIX = ROWS * W             # 262144 valid pixels per core
G = 6
P = G * C                   # 114 partitions
TILE_F = 4096
NT = 11
F = NT * TILE_F             # 45056
NPAD = G * F - NPIX         # 8192 pad pixels
VALID_J5 = NPIX - 5 * F     # 36864 valid pixels in group 5
PAD_TILE0 = VALID_J5 // TILE_F  # = 9; tiles 9,10 have group 5 all-pad

THR = [np.float32(i / 10.0) for i in range(10)]
N_CONF = 10
N_CNT = 9
NFOLD = N_CONF + N_CNT      # 19 fold slots per tile
DVE_FOLD_CONF = 3           # conf folds 0..2 on DVE, 3..9 on ACT

MM_CHUNK = 512              # fp32 moving-operand limit
PCOLS = NT * 256            # pixel-major cols: (t, h, c128) -> t*256+h*128+c

_BUILD_CACHE = {}


def _patch_tile_drain():
    """walrus rejects drains with >1 sync wait; split the tile-exit drain."""
    if getattr(tile.TileContext, "_drain_patched", False):
        return

    def _drain_and_barrier(self, tick_clock, wait_clock):
        drain_inst = self.nc.sync.drain()
        wait_clock.add_sem_waits(
            drain_inst.ins, ScopedClock({None: tick_clock.global_clock})
        )
        si = drain_inst.ins.sync_info
        if si is not None and len(si.on_wait) > 1:
            waits = list(si.on_wait)
            ups = list(si.on_update)
            drain_inst.ins.sync_info = mybir.SyncInfo(on_wait=waits[:1], on_update=[])
            last = drain_inst
            for i in range(1, len(waits)):
                last = self.nc.sync.drain()
                last.ins.sync_info = mybir.SyncInfo(on_wait=waits[i:i + 1], on_update=[])
            if ups:
                lw = list(last.ins.sync_info.on_wait) if last.ins.sync_info else []
                last.ins.sync_info = mybir.SyncInfo(on_wait=lw, on_update=ups)
        self.nc.all_engine_barrier()
        assert self.sems is not None
        popped = self.nc._tile_sem_poison_stack.pop()
        assert popped is self._sem_poison
        self.nc.clear_and_free_semaphores(list(self.sems.allocated().values()))
        self.nc.all_engine_barrier()

    tile.TileContext._drain_and_barrier = _drain_and_barrier
    tile.TileContext._drain_patched = True


def build_nc():
    _patch_tile_drain()
    nc = bacc.Bacc()

    # register threshold constants (+/-t_i) as const APs
    for i in range(1, 10):
        for v in (float(-THR[i]), float(THR[i])):
            if (F32, v) in nc.const_aps.aps:
                continue
            tns = nc.alloc_sbuf_tensor(f"const-thr-{v}", [128, 1], F32)
            nc.gpsimd.memset(tns.ap(), v)
            nc.const_aps.aps[(F32, v)] = tns.ap()
    nc.all_engine_barrier()

    lg = nc.declare_dram_parameter("lg", [C, NPIX], F32, isOutput=False)
    zpad = nc.declare_dram_parameter("zpad", [C, TILE_F], F32, isOutput=False)
    lstar = nc.declare_dram_parameter("lstar", [96, PCOLS], F32, isOutput=False)
    bdones = nc.declare_dram_parameter("bdones", [P, G], F32, isOutput=False)
    bcast = nc.declare_dram_parameter("bcast", [G, P], F32, isOutput=False)
    folds_out = nc.declare_dram_parameter("folds", [P, NT * NFOLD], F32, isOutput=True)
    pstar_out = nc.declare_dram_parameter("pstar", [96, PCOLS], F32, isOutput=True)

    with tile.TileContext(nc) as tc:
        with (
            tc.tile_pool(name="const", bufs=1) as constp,
            tc.tile_pool(name="lt", bufs=2) as lp,
            tc.tile_pool(name="et", bufs=2) as ep,
            tc.tile_pool(name="pt", bufs=2) as pp,
            tc.tile_pool(name="mc", bufs=2) as mcp,
            tc.tile_pool(name="mt", bufs=2) as mp,
            tc.tile_pool(name="lst", bufs=2) as lsp,
            tc.tile_pool(name="acc", bufs=1) as accp,
            tc.tile_pool(name="zpsum", bufs=1, space="PSUM") as zp,
            tc.tile_pool(name="mbpsum", bufs=2, space="PSUM") as mbp,
        ):
            bd_sb = constp.tile([P, G], F32)
            nc.gpsimd.dma_start(out=bd_sb[:], in_=bdones[:])
            bc_sb = constp.tile([G, P], F32)
            nc.gpsimd.dma_start(out=bc_sb[:], in_=bcast[:])

            foldacc = accp.tile([P, NT * NFOLD], F32)
            dstar = accp.tile([96, PCOLS], F32)
            scr_dve = accp.tile([P, TILE_F], F32)
            scr_act = accp.tile([P, TILE_F], F32)

            for t in range(NT):
                # ---- load logits tile [114, TILE_F] ----
                lt = lp.tile([P, TILE_F], F32)
                ng = G if t < PAD_TILE0 else G - 1
                base = lg[:, t * TILE_F:(t + 1) * TILE_F]
                src3 = bass_rust.AP(
                    tensor=base.tensor, offset=base.offset,
                    ap=[[F, ng]] + list(base.ap))
                nc.gpsimd.dma_start(out=lt[0:C * ng, :], in_=src3)
                if ng < G:
                    nc.gpsimd.dma_start(out=lt[C * 5:P, :], in_=zpad[:])

                # ---- e = exp(l) ----
                et = ep.tile([P, TILE_F], F32)
                nc.scalar.activation(et[:], lt[:], AF.Exp)

                # ---- l* tile (pixel-major) ----
                lst = lsp.tile([96, 256], F32)
                nc.gpsimd.dma_start(out=lst[:], in_=lstar[:, t * 256:(t + 1) * 256])

                # ---- per 2048-half: Z, ln, broadcast, d = l - mb ----
                for h in range(2):
                    zps = zp.tile([G, 2048], F32)
                    for q in range(4):
                        c0 = h * 2048 + q * MM_CHUNK
                        nc.tensor.matmul(
                            zps[:, q * MM_CHUNK:(q + 1) * MM_CHUNK],
                            bd_sb[:],
                            et[:, c0:c0 + MM_CHUNK],
                            start=True, stop=True,
                        )
                    # m = ln(Z): ACT reads PSUM directly, writes [6, 2048]
                    mt = mp.tile([G, 2048], F32)
                    nc.scalar.activation(mt[:], zps[:], AF.Ln)
                    # pixel-major copy of m for the true-class side channel
                    mc = mcp.tile([96, 128], F32)
                    nc.gpsimd.dma_start(
                        out=mc[:],
                        in_=mt[:].rearrange("g (r c) -> g r c", r=16),
                    )
                    # d* = l* - m  (pixel-major [96, 128])
                    nc.vector.tensor_sub(
                        dstar[:, t * 256 + h * 128:t * 256 + (h + 1) * 128],
                        lst[:, h * 128:(h + 1) * 128],
                        mc[:],
                    )
                    for hh in range(2):
                        mb = mbp.tile([P, 1024], F32)
                        for q in range(2):
                            c0 = hh * 1024 + q * MM_CHUNK
                            nc.tensor.matmul(
                                mb[:, q * MM_CHUNK:(q + 1) * MM_CHUNK],
                                bc_sb[:],
                                mt[:, c0:c0 + MM_CHUNK],
                                start=True, stop=True,
                            )
                        d0 = h * 2048 + hh * 1024
                        nc.vector.tensor_sub(
                            lt[:, d0:d0 + 1024], lt[:, d0:d0 + 1024], mb[:],
                        )

                # ---- p = exp(d) ----
                pt = pp.tile([P, TILE_F], F32)
                nc.scalar.activation(pt[:], lt[:], AF.Exp)

                # ---- folds ----
                base = t * NFOLD
                for i in range(1, 10):   # counts on DVE: accum = sum [p > t]
                    nc.vector.tensor_scalar(
                        scr_dve[:], pt[:], float(THR[i]), None, ALU.is_gt,
                        ALU.add,
                        accum_out=foldacc[:, base + N_CONF + i - 1:base + N_CONF + i],
                    )
                for i in range(10):      # conf folds
                    col = foldacc[:, base + i:base + i + 1]
                    if i == 0:
                        # accum = sum max(p, 0) = sum p
                        nc.vector.tensor_scalar(
                            scr_dve[:], pt[:], 0.0, None,
                            ALU.max, ALU.add, accum_out=col,
                        )
                    elif i < DVE_FOLD_CONF:
                        # accum = sum (max(p, t) - t) = sum relu(p - t)
                        nc.vector.scalar_tensor_tensor(
                            scr_dve[:], pt[:], float(THR[i]),
                            nc.const_aps.tensor(float(THR[i]), [P, TILE_F]),
                            ALU.max, ALU.subtract, accum_out=col,
                        )
                    else:
                        nc.scalar.activation(
                            scr_act[:], pt[:], AF.Relu,
                            bias=-float(THR[i]), accum_out=col,
                        )

            # ---- end phase ----
            pstar_sb = accp.tile([96, PCOLS], F32)
            nc.scalar.activation(pstar_sb[:], dstar[:], AF.Exp)
            nc.gpsimd.dma_start(out=pstar_out[:], in_=pstar_sb[:])
            nc.gpsimd.dma_start(out=folds_out[:], in_=foldacc[:])

    nc.finalize()
    return nc


def _make_consts():
    bd = np.zeros((P, G), np.float32)
    bc = np.zeros((G, P), np.float32)
    for g in range(G):
        bd[C * g:C * (g + 1), g] = 1.0
        bc[g, C * g:C * (g + 1)] = 1.0
    return bd, bc


def _shard_host(output: np.ndarray, target: np.ndarray):
    o = np.ascontiguousarray(output[0])          # [19, 1024, 2048]
    t = np.ascontiguousarray(target[0])          # [1024, 2048]
    lstar_full = np.take_along_axis(o, t[None], axis=0)[0]
    bd, bc = _make_consts()

    in_maps = []
    for core in range(NCORES):
        r0 = core * ROWS
        lg = np.ascontiguousarray(o[:, r0:r0 + ROWS, :].reshape(C, NPIX))
        ls = lstar_full[r0:r0 + ROWS, :].reshape(-1)
        ls = np.concatenate([ls, np.zeros(NPAD, np.float32)])
        # flat n = g*F + t*4096 + h*2048 + r*128 + c  ->  [96=(g,r), t*256+h*128+c]
        ls = (ls.reshape(G, NT, 2, 16, 128).transpose(0, 3, 1, 2, 4)
                .reshape(96, PCOLS))
        in_maps.append({
            "lg": lg, "lstar": np.ascontiguousarray(ls),
            "bdones": bd, "bcast": bc,
            "zpad": np.zeros((C, TILE_F), np.float32),
        })
    return in_maps


def _pstar_to_flat(ps: np.ndarray) -> np.ndarray:
    """[96, PCOLS] pixel-major -> core-flat [G*F] (incl pad)."""
    return (ps.reshape(G, 16, NT, 2, 128).transpose(0, 2, 3, 1, 4).reshape(-1))


def _decode_and_loss(results, target: np.ndarray):
    conf = np.zeros((C, NB), np.float64)
    cnt = np.zeros((C, NB), np.float64)
    acc = np.zeros((C, NB), np.float64)

    # device-replicated pad probability: p_pad = exp(0 - ln(19*exp(0)))
    p_pad = np.float32(np.exp(np.float32(-np.log(np.float32(19.0)))))

    for core in range(NCORES):
        folds = results[core]["folds"].astype(np.float64)
        folds = folds.reshape(P, NT, NFOLD).sum(axis=1)      # [114, 19]
        R = folds[:, :N_CONF].reshape(G, C, N_CONF).sum(axis=0)   # [C, 10]
        Ni = folds[:, N_CONF:].reshape(G, C, N_CNT).sum(axis=0)   # [C, 9]

        R[:, 0] -= NPAD * np.float64(p_pad)      # pad contributes only to R_0
        Ni = np.concatenate([np.full((C, 1), float(NPIX)), Ni], axis=1)

        tgrid = np.arange(10, dtype=np.float64) / 10.0
        S = R + tgrid[None, :] * Ni              # S_i = sum p * [p > t_i]
        Snext = np.concatenate([S[:, 1:], np.zeros((C, 1))], axis=1)
        Nnext = np.concatenate([Ni[:, 1:], np.zeros((C, 1))], axis=1)
        conf += S - Snext
        cnt += Ni - Nnext

        r0 = core * ROWS
        ps = _pstar_to_flat(results[core]["pstar"])[:NPIX]
        y = target[0, r0:r0 + ROWS, :].reshape(-1)
        b = np.clip(np.ceil(ps * np.float32(10.0)).astype(np.int32) - 1, 0, NB - 1)
        acc += np.bincount(y * NB + b, minlength=C * NB).reshape(C, NB)

    EPS = 1e-13
    avg_acc = acc / (cnt + EPS)
    avg_conf = conf / (cnt + EPS)
    loss = np.sum((avg_acc - avg_conf) ** 2 * (cnt / cnt.sum()))
    return np.float32(loss), (conf, cnt, acc)


def kernel(output: np.ndarray, target: np.ndarray) -> np.ndarray:
    output = np.asarray(output, np.float32)
    target = np.asarray(target, np.int32)
    if "nc" not in _BUILD_CACHE:
        _BUILD_CACHE["nc"] = build_nc()
    nc = _BUILD_CACHE["nc"]
    in_maps = _shard_host(output, target)
    res = run_bass_kernel_spmd(nc, in_maps, list(range(NCORES)))
    loss, _ = _decode_and_loss(res.results, target)
    return np.float32(loss)



# revision 10
# speedup vs baseline: 1.5497x; 1.5497x over previous
"""Trainium2 Bass kernel for nn_CCELoss (calibration-histogram loss), v2.

Sharding: data-parallel over image rows, 8 NeuronCores, 128 rows each.

Per-core layout: logits as [114 = 6 pixel-groups x 19 classes, F=45056]
(group g covers core-flat pixels [g*F, (g+1)*F)). NPIX=262144 valid pixels;
the tail of group 5 (tiles 9,10) is excluded from folds entirely (partitions
[95:114) unwritten there), so no pad corrections are needed.

Per 4096-pixel tile:
  ACT  e = exp(l)                  fp32 -> fp16
  PE   Z[g,n] = sum_c e[(g,c),n]   (block-diag ones matmul, fp16 -> PSUM f32)
  DMA  reshape Z [6,2048] -> [96,128] pixel-major (x2 halves)
  ACT  m = ln(Z)  on [96,256]      (cheap: 256 cols instead of 4096)
  DVE  d* = l* - m                 (pixel-major [96,256], true-class channel)
  DMA  m [96,256] -> lt[114:120]   (m joins the logit tile as 6 extra rows)
  PE   d = DM.T @ [l; m; l*]       (fp32 matmul: d = l - m[g], d* = l* - m, PSUM)
  ACT  p = exp(d)                  PSUM -> SBUF fp16
  19 folds on p (fp16, fp32 accumulation), one op each:
    counts N_i = sum [p > i/10]            (is_gt / add-reduce)
    S'_i   = sum max(p, i/10) - 4096*i/10  (max / add-reduce, scalar2 post-add)
    split DVE (fast 2-byte mode) / ACT (Relu bias, Sign) / GPSIMD
End: pstar = exp(dstar) on [96, 2816], DMA out (host bins accuracy histogram).
Host: decode S/N -> conf/cnt hists, bin p* -> acc hist, loss formula.
"""

import numpy as np

import bass_rust
import concourse.bass as bass
from concourse import bacc
import concourse.mybir as mybir
import concourse.tile as tile
from concourse.vector_clock import ScopedClock
from concourse.bass_utils import run_bass_kernel_spmd

F32 = mybir.dt.float32
F32R = mybir.dt.float32r
F16 = mybir.dt.float16
AF = mybir.ActivationFunctionType
ALU = mybir.AluOpType

# ---------------- problem geometry (hardcoded) ----------------
C = 19
NB = 10
H, W = 1024, 2048
NCORES = 8
ROWS = H // NCORES          # 128
NPIX = ROWS * W             # 262144 valid pixels per core
G = 6
P = G * C                   # 114 partitions of logits
PM = P + G                  # 120 partitions incl. m rows
PML = PM + G                # 126 partitions incl. l* rows
TILE_F = 4096
NT = 11
F = NT * TILE_F             # 45056
VALID_J5 = NPIX - 5 * F     # 36864 valid pixels in group 5
PAD_TILE0 = VALID_J5 // TILE_F  # = 9; tiles 9,10 have group 5 all-pad

THR = [np.float32(i / 10.0) for i in range(10)]
NFOLD = 19                  # 9 counts (i=1..9) + 10 conf (i=0..9)
MM_CHUNK = 512

# fold engine assignment (tensor_scalar is not a legal GPSIMD opcode)
DVE_FOLDS = [("cnt", i) for i in range(1, 10)] + [("conf", i) for i in range(0, 8)]
ACT_FOLDS = [("conf", 8), ("conf", 9)]
NPAIR = NT // 2 + NT % 2    # fold groups: (0,1)..(8,9),(10)

_BUILD_CACHE = {}


def _patch_tile_drain():
    """walrus rejects drains with >1 sync wait; split the tile-exit drain."""
    if getattr(tile.TileContext, "_drain_patched", False):
        return

    def _drain_and_barrier(self, tick_clock, wait_clock):
        drain_inst = self.nc.sync.drain()
        wait_clock.add_sem_waits(
            drain_inst.ins, ScopedClock({None: tick_clock.global_clock})
        )
        si = drain_inst.ins.sync_info
        if si is not None and len(si.on_wait) > 1:
            waits = list(si.on_wait)
            ups = list(si.on_update)
            drain_inst.ins.sync_info = mybir.SyncInfo(on_wait=waits[:1], on_update=[])
            last = drain_inst
            for i in range(1, len(waits)):
                last = self.nc.sync.drain()
                last.ins.sync_info = mybir.SyncInfo(on_wait=waits[i:i + 1], on_update=[])
            if ups:
                lw = list(last.ins.sync_info.on_wait) if last.ins.sync_info else []
                last.ins.sync_info = mybir.SyncInfo(on_wait=lw, on_update=ups)
        self.nc.all_engine_barrier()
        assert self.sems is not None
        popped = self.nc._tile_sem_poison_stack.pop()
        assert popped is self._sem_poison
        self.nc.clear_and_free_semaphores(list(self.sems.allocated().values()))
        self.nc.all_engine_barrier()

    tile.TileContext._drain_and_barrier = _drain_and_barrier
    tile.TileContext._drain_patched = True


def _fold_slot(kind, i):
    # per-tile slot layout: counts i=1..9 -> 0..8; conf i=0..9 -> 9..18
    return (i - 1) if kind == "cnt" else (9 + i)


def build_nc():
    _patch_tile_drain()
    nc = bacc.Bacc()

    # const APs for ACT fold biases
    for kind, i in ACT_FOLDS:
        v = float(-THR[i])
        if (F32, v) not in nc.const_aps.aps:
            tns = nc.alloc_sbuf_tensor(f"const-b{i}", [128, 1], F32)
            nc.gpsimd.memset(tns.ap(), v)
            nc.const_aps.aps[(F32, v)] = tns.ap()
    nc.all_engine_barrier()

    lg = nc.declare_dram_parameter("lg", [C, NPIX], F32, isOutput=False)
    lstar = nc.declare_dram_parameter("lstar", [G, F], F32, isOutput=False)
    bd16 = nc.declare_dram_parameter("bd16", [P, G], F16, isOutput=False)
    dmat = nc.declare_dram_parameter("dmat", [PML, PM], F32, isOutput=False)
    folds_out = nc.declare_dram_parameter("folds", [P, NPAIR * NFOLD], F32, isOutput=True)
    pstar_out = nc.declare_dram_parameter("pstar", [G, F], F16, isOutput=True)

    with tile.TileContext(nc) as tc:
        with (
            tc.tile_pool(name="const", bufs=1) as constp,
            tc.tile_pool(name="lt", bufs=2) as lp,
            tc.tile_pool(name="et", bufs=2) as ep,
            tc.tile_pool(name="pt", bufs=2) as pp,
            tc.tile_pool(name="mt", bufs=2) as mp,
            tc.tile_pool(name="acc", bufs=1) as accp,
            tc.tile_pool(name="zpsum", bufs=1, space="PSUM") as zp,
            tc.tile_pool(name="dpsum", bufs=1, space="PSUM") as dp,
        ):
            bd_sb = constp.tile([P, G], F16)
            nc.gpsimd.dma_start(out=bd_sb[:], in_=bd16[:])
            dm_sb = constp.tile([PML, PM], F32)
            nc.gpsimd.dma_start(out=dm_sb[:], in_=dmat[:])

            foldacc = accp.tile([P, NPAIR * NFOLD], F32)
            nc.gpsimd.memset(foldacc[:], 0.0)
            scr_dve = accp.tile([P, 2 * TILE_F], F16)
            scr_act = accp.tile([P, 2 * TILE_F], F16)

            for t in range(NT):
                pad = t >= PAD_TILE0
                ng = G - 1 if pad else G
                Pr = C * ng

                # ---- load logits tile rows [0:Pr] ----
                lt = lp.tile([PML, TILE_F], F32)
                if pad:
                    # pad columns get logits [0, -80 x18] on the group-5 rows
                    # -> p = [1, 0 x18] exactly; folded uniformly, corrected on
                    # host. (engine partition offsets must be 32-aligned; the
                    # DMA below overwrites rows [64:95) with real logits.)
                    nc.gpsimd.memset(lt[64:96, :], 0.0)
                    nc.gpsimd.memset(lt[96:P, :], -80.0)
                base = lg[:, t * TILE_F:(t + 1) * TILE_F]
                src3 = bass_rust.AP(
                    tensor=base.tensor, offset=base.offset,
                    ap=[[F, ng]] + list(base.ap))
                nc.gpsimd.dma_start(out=lt[0:Pr, :], in_=src3)
                # l* rows [120:126]
                nc.gpsimd.dma_start(
                    out=lt[PM:PML, :],
                    in_=lstar[:, t * TILE_F:(t + 1) * TILE_F])

                # ---- e = exp(l) -> fp16 ----
                et = ep.tile([P, TILE_F], F16)
                nc.scalar.activation(et[:], lt[0:P, :], AF.Exp)

                # ---- Z per half -> mt via Ln, then DMA into lt[114:120] ----
                mt = mp.tile([G, TILE_F], F32)
                for h in range(2):
                    zps = zp.tile([G, 2048], F32)
                    for q in range(4):
                        c0 = h * 2048 + q * MM_CHUNK
                        nc.tensor.matmul(
                            zps[:, q * MM_CHUNK:(q + 1) * MM_CHUNK],
                            bd_sb[:],
                            et[:, c0:c0 + MM_CHUNK],
                            start=True, stop=True,
                        )
                    nc.scalar.activation(
                        mt[:, h * 2048:(h + 1) * 2048], zps[:], AF.Ln)
                nc.gpsimd.dma_start(out=lt[P:PM, :], in_=mt[:])

                # ---- d = DM.T @ [l; m; l*] (fp32), p/p* = exp(d) ----
                if t % 2 == 0:
                    pt = pp.tile([PM, 2 * TILE_F], F16)
                pc0 = (t % 2) * TILE_F
                for h in range(2):
                    dps = dp.tile([PM, 2048], F32)
                    for s in range(4):
                        c0 = h * 2048 + s * MM_CHUNK
                        nc.tensor.matmul(
                            dps[:, s * MM_CHUNK:(s + 1) * MM_CHUNK],
                            dm_sb[:],
                            lt[:, c0:c0 + MM_CHUNK],
                            start=True, stop=True,
                        )
                    nc.scalar.activation(
                        pt[:, pc0 + h * 2048:pc0 + (h + 1) * 2048], dps[:], AF.Exp)

                # ---- p* rows out ----
                nc.gpsimd.dma_start(
                    out=pstar_out[:, t * TILE_F:(t + 1) * TILE_F],
                    in_=pt[P:PM, pc0:pc0 + TILE_F])

                # ---- folds: on tile pairs (and the final single tile) ----
                if t % 2 == 1 or t == NT - 1:
                    wf = TILE_F if t == NT - 1 and t % 2 == 0 else 2 * TILE_F
                    grp = t // 2
                    fb = foldacc[:, grp * NFOLD:(grp + 1) * NFOLD]
                    for kind, i in DVE_FOLDS:
                        s = _fold_slot(kind, i)
                        op0 = ALU.is_gt if kind == "cnt" else ALU.max
                        nc.vector.tensor_scalar(
                            scr_dve[:, 0:wf], pt[0:P, 0:wf], float(THR[i]), None,
                            op0, ALU.add, accum_out=fb[:, s:s + 1])
                    for kind, i in ACT_FOLDS:
                        s = _fold_slot(kind, i)
                        nc.scalar.activation(
                            scr_act[:, 0:wf], pt[0:P, 0:wf], AF.Relu,
                            bias=float(-THR[i]), accum_out=fb[:, s:s + 1])

            # ---- end phase ----
            nc.gpsimd.dma_start(out=folds_out[:], in_=foldacc[:])

    nc.finalize()
    return nc


def _make_consts():
    bd = np.zeros((P, G), np.float16)
    dm = np.zeros((PML, PM), np.float32)
    for g in range(G):
        bd[C * g:C * (g + 1), g] = 1.0
    for k in range(P):
        dm[k, k] = 1.0
        dm[P + k // C, k] = -1.0
    for g in range(G):
        dm[PM + g, P + g] = 1.0
        dm[P + g, P + g] = -1.0
    return bd, dm


def _shard_host(output: np.ndarray, target: np.ndarray):
    o = np.ascontiguousarray(output[0])          # [19, 1024, 2048]
    t = np.ascontiguousarray(target[0])          # [1024, 2048]
    lstar_full = np.take_along_axis(o, t[None], axis=0)[0]
    bd, dm = _make_consts()

    NPAD = G * F - NPIX
    in_maps = []
    for core in range(NCORES):
        r0 = core * ROWS
        lgc = np.ascontiguousarray(o[:, r0:r0 + ROWS, :].reshape(C, NPIX))
        ls = lstar_full[r0:r0 + ROWS, :].reshape(-1)
        ls = np.concatenate([ls, np.zeros(NPAD, np.float32)]).reshape(G, F)
        in_maps.append({
            "lg": lgc, "lstar": np.ascontiguousarray(ls),
            "bd16": bd, "dmat": dm,
        })
    return in_maps


def _decode_and_loss(results, target: np.ndarray):
    conf = np.zeros((C, NB), np.float64)
    cnt = np.zeros((C, NB), np.float64)
    acc = np.zeros((C, NB), np.float64)
    tgrid = np.arange(10, dtype=np.float64) / 10.0
    act_slots = {i for (k, i) in ACT_FOLDS}

    PADCOLS = 2 * TILE_F      # 8192 pad cols per class-row (tiles 9,10)
    for core in range(NCORES):
        folds = results[core]["folds"].astype(np.float64)
        folds = folds.reshape(P, NPAIR, NFOLD).sum(axis=1)        # [114, 19]
        folds = folds.reshape(G, C, NFOLD).sum(axis=0)            # [C, 19]
        Ncnt = folds[:, 0:9]                                      # [C, 9] i=1..9
        M = folds[:, 9:19]                                        # [C, 10]
        # pad corrections: pad columns contribute p=1 on class 0, p=0 on 1..18
        Ncnt[0, :] -= PADCOLS
        for i in range(10):
            if i in act_slots:        # ACT Relu fold: relu(1-t) on class 0
                M[0, i] -= PADCOLS * (1.0 - tgrid[i])
            else:                     # max fold: max(1,t)=1 cls0; max(0,t)=t rest
                M[0, i] -= PADCOLS * 1.0
                M[1:, i] -= PADCOLS * tgrid[i]
        Ni = np.concatenate(
            [np.full((C, 1), float(NPIX)), Ncnt], axis=1)            # [C, 10]
        # max-form conf folds accumulated sum(max(p,t)) over NPIX valid cols;
        # R = M - t*NPIX.  ACT Relu folds are already R.
        R = np.empty_like(M)
        for i in range(10):
            R[:, i] = M[:, i] if i in act_slots else M[:, i] - tgrid[i] * NPIX

        S = R + tgrid[None, :] * Ni              # S_i = sum p * [p > t_i]
        Snext = np.concatenate([S[:, 1:], np.zeros((C, 1))], axis=1)
        Nnext = np.concatenate([Ni[:, 1:], np.zeros((C, 1))], axis=1)
        conf += S - Snext
        cnt += Ni - Nnext

        r0 = core * ROWS
        ps = results[core]["pstar"].astype(np.float32).reshape(-1)[:NPIX]
        y = target[0, r0:r0 + ROWS, :].reshape(-1)
        b = np.clip(np.ceil(ps * np.float32(10.0)).astype(np.int32) - 1, 0, NB - 1)
        acc += np.bincount(y * NB + b, minlength=C * NB).reshape(C, NB)

    EPS = 1e-13
    avg_acc = acc / (cnt + EPS)
    avg_conf = conf / (cnt + EPS)
    loss = np.sum((avg_acc - avg_conf) ** 2 * (cnt / cnt.sum()))
    return np.float32(loss), (conf, cnt, acc)


def kernel(output: np.ndarray, target: np.ndarray) -> np.ndarray:
    output = np.asarray(output, np.float32)
    target = np.asarray(target, np.int32)
    if "nc" not in _BUILD_CACHE:
        _BUILD_CACHE["nc"] = build_nc()
    nc = _BUILD_CACHE["nc"]
    in_maps = _shard_host(output, target)
    res = run_bass_kernel_spmd(nc, in_maps, list(range(NCORES)))
    loss, _ = _decode_and_loss(res.results, target)
    return np.float32(loss)
